# revision 7
# baseline (speedup 1.0000x reference)
"""Trainium2 Bass kernel for a 4-layer compressed model:

    for l in range(4):  x = x @ (base[l] + bitdelta[l] * mask[l])

x: [16, 4096] f32, base/mask: [4, 4096, 4096] f32, bitdelta: [4] f32.

Sharding (8 cores, tensor parallel on weight columns):
  core c owns columns [c*512, (c+1)*512) of every layer's weight.

Key ideas:
  * Weights are never reconstructed anywhere: by linearity,
        x @ (base + bd*mask) = x @ base + (bd*x) @ mask,
    so the PE accumulates both streams into ONE PSUM bank, using two
    stationary operands: x^T and a pre-scaled bd*x^T (one tiny scaled
    copy on the scalar engine per layer). No DVE pass over the weights,
    no fixup after the accumulation.
  * Low-precision streaming: base is cast to bf16 on the host (quant
    noise ~0.2% of base, itself ~25x smaller than bd*mask); mask is
    +/-1 exactly, which fp8e4m3 represents exactly. Activations ride
    in bf16. HBM traffic per core: 64 MiB (f32) -> 24 MiB. bitdelta
    values are baked into the program as immediates (compiled after
    inputs are known; cache keyed on them).
  * Contraction order is permuted to d = p*32 + k (p = SBUF partition,
    k = matmul index). The host lays weight shards out so every 1 MiB
    weight DMA is one fully contiguous DRAM block, and the activation
    x^T [4096, 16] loads land partition-contiguous.
  * Between layers the [16,512] local result is PE-transposed to
    [512,16] bf16 and AllGather'd on the partition axis into the next
    layer's x^T. Three layers of weights are buffered in SBUF so the
    gather latency hides behind the weight stream.
  * During each gather window a chain of throwaway matmuls keeps the
    PE busy so the HAM clock gate stays at 2.4 GHz: cold restarts both
    slow the next layer 2x and skew cores apart, which inflates every
    collective (completion = max over ranks).

Memory-bound: each core streams 24 MiB of weights; roofline ~70 us.
"""

import numpy as np

import concourse.bass as bass
import concourse.mybir as mybir
import concourse.tile as tile
from concourse import bacc
from concourse.bass_utils import run_bass_kernel_spmd
from concourse.masks import make_identity

L = 4
D = 4096
B = 16
NCORES = 8
C = D // NCORES          # 512 columns per core
KT = D // 128            # 32 contraction tiles of 128
GKB = 8                  # k-tiles per base DMA (1 MiB bf16 transfers)
NGB = KT // GKB          # 4 base DMAs per layer
GKM = 16                 # k-tiles per mask DMA (1 MiB fp8 transfers)
NGM = KT // GKM          # 2 mask DMAs per layer
XCH = 4                  # x^T load chunks per layer
KXC = KT // XCH          # k-tiles per x chunk
CT = C // 128            # 4 transpose chunks
WBUFS_B = 12             # base tiles in flight (12 MiB, 3 layers)
WBUFS_M = 6              # mask tiles in flight (6 MiB, 3 layers)
NWARM = 32               # PE-warmer matmuls per gather window

F32 = mybir.dt.float32
BF16 = mybir.dt.bfloat16
FP8 = mybir.dt.float8e4
ALU = mybir.AluOpType
ACT = mybir.ActivationFunctionType

_cache = {}


def build(bd_vals):
    nc = bacc.Bacc(
        "TRN2",
        target_bir_lowering=False,
        debug=False,
        num_devices=NCORES,
    )

    # x^T in natural [4096, 16] order; row d = p*KT + k maps to SBUF
    # partition p, matmul index k — so the load is partition-contiguous.
    xT0 = nc.dram_tensor("xT0", [D, B], BF16, kind="ExternalInput")
    # weight shards, pre-permuted on host: [l, g, p, j*C+c] = W_l[p*KT+g*GK+j,
    # c]; each [128, GK*C] block is 1 MiB contiguous.
    base_sh = nc.dram_tensor("base_sh", [L, NGB, 128, GKB * C], BF16,
                             kind="ExternalInput")
    mask_sh = nc.dram_tensor("mask_sh", [L, NGM, 128, GKM * C], FP8,
                             kind="ExternalInput")
    out = nc.dram_tensor("out", [B, C], F32, kind="ExternalOutput")

    rg = [list(range(NCORES))]

    def load_xt_chunks(xpool, src):
        """Load x^T [D, B] into 4 SBUF chunk tiles of 8 k-tiles each."""
        chunks = []
        for xc in range(XCH):
            xt = xpool.tile([128, KXC * B], BF16, tag=f"xt{xc}")
            nc.scalar.dma_start(
                xt[:, :].rearrange("p (k b) -> p k b", k=KXC),
                src[:, :].rearrange("(p k) b -> p k b", p=128)
                [:, xc * KXC:(xc + 1) * KXC],
            )
            chunks.append(xt)
        return chunks

    with tile.TileContext(nc) as tc:
        with (
            tc.tile_pool(name="wb", bufs=WBUFS_B) as bpool,
            tc.tile_pool(name="wm", bufs=WBUFS_M) as mpool,
            tc.tile_pool(name="xp", bufs=2) as xpool,
            tc.tile_pool(name="sp", bufs=2) as spool,
            tc.tile_pool(name="const", bufs=1) as cpool,
            tc.tile_pool(name="acc", bufs=2, space="PSUM") as psum,
            tc.tile_pool(name="tp", bufs=4, space="PSUM") as tpsum,
            tc.tile_pool(name="warm", bufs=1, space="PSUM") as wpsum,
            tc.tile_pool(name="dram", bufs=2, space="DRAM") as dram,
        ):
            ident = cpool.tile([B, B], F32, tag="ident")
            make_identity(nc, ident[:, :])
            wl = cpool.tile([128, B], BF16, tag="wl")
            nc.vector.memset(wl[:, :], 0.0)

            # Issue every weight DMA up front; the 3-layer-deep pools let
            # the stream run ahead of compute continuously.
            wbs, wms = [], []
            for l in range(L):
                wbs.append([])
                wms.append([])
                for g in range(NGM):
                    wm = mpool.tile([128, GKM * C], FP8, tag="wm")
                    nc.sync.dma_start(wm[:, :], mask_sh[l, g])
                    wms[l].append(wm)
                for g in range(NGB):
                    wb = bpool.tile([128, GKB * C], BF16, tag="wb")
                    nc.sync.dma_start(wb[:, :], base_sh[l, g])
                    wbs[l].append(wb)

            xts = load_xt_chunks(xpool, xT0)

            for l in range(L):
                bd = float(bd_vals[l])
                # xs = bd * x^T: the mask stream's stationary operand.
                xss = []
                for xc in range(XCH):
                    xs = xpool.tile([128, KXC * B], BF16, tag=f"xs{xc}")
                    nc.scalar.activation(xs[:, :], xts[xc][:, :], ACT.Copy,
                                         scale=bd)
                    xss.append(xs)

                acc = psum.tile([B, C], F32, tag="acc")
                for k in range(KT):
                    xc, kk = k // KXC, k % KXC
                    jb, jm = k % GKB, k % GKM
                    nc.tensor.matmul(
                        acc[:, :],
                        xts[xc][:, kk * B:(kk + 1) * B],
                        wbs[l][k // GKB][:, jb * C:(jb + 1) * C],
                        start=(k == 0),
                        stop=False,
                    )
                    nc.tensor.matmul(
                        acc[:, :],
                        xss[xc][:, kk * B:(kk + 1) * B],
                        wms[l][k // GKM][:, jm * C:(jm + 1) * C],
                        start=False,
                        stop=(k == KT - 1),
                    )

                y_sb = spool.tile([B, C], F32, tag="y")
                nc.scalar.copy(y_sb[:, :], acc[:, :])

                if l == L - 1:
                    nc.scalar.dma_start(out[:, :], y_sb[:, :])
                else:
                    # y [16, 512] -> y^T [512, 16] via 4 PE transposes,
                    # then AllGather into the next layer's x^T [4096, 16].
                    yt_sb = spool.tile([128, CT * B], BF16, tag="yt")
                    for cc in range(CT):
                        pt = tpsum.tile([128, B], F32, tag="pt")
                        nc.tensor.transpose(
                            pt[:, :],
                            y_sb[:, cc * 128:(cc + 1) * 128],
                            ident[:, :],
                        )
                        nc.scalar.copy(
                            yt_sb[:, cc * B:(cc + 1) * B], pt[:, :]
                        )
                    ytb = dram.tile([C, B], BF16, tag="ytb")
                    nc.sync.dma_start(
                        ytb[:, :].rearrange("(cc p) b -> p cc b", p=128),
                        yt_sb[:, :].rearrange("p (cc b) -> p cc b", cc=CT),
                    )
                    xt_full = dram.tile([D, B], BF16, tag="xtf",
                                        addr_space="Shared")
                    nc.gpsimd.collective_compute(
                        "AllGather",
                        ALU.bypass,
                        replica_groups=rg,
                        ins=[ytb.opt()],
                        outs=[xt_full.opt()],
                    )

                    # PE warmers: a chain of throwaway matmuls on resident
                    # data that runs back-to-back while the gather is in
                    # flight, holding the HAM clock gate open.
                    warm_ps = wpsum.tile([B, C], F32, tag="warm")
                    wsrc = wbs[l + 1][0]
                    for i in range(NWARM):
                        nc.tensor.matmul(
                            warm_ps[:, :],
                            wl[:, :],
                            wsrc[:, :C],
                            start=(i == 0),
                            stop=(i == NWARM - 1),
                        )

                    xts = load_xt_chunks(xpool, xt_full)

    nc.compile()
    return nc


def _get_nc(bd_vals):
    key = tuple(float(v) for v in bd_vals)
    if _cache.get("key") != key:
        _cache["nc"] = build(bd_vals)
        _cache["key"] = key
    return _cache["nc"]


def _shard_weight(w, gk):
    """[L, D, C] column shard -> [L, KT//gk, 128, gk*C] with
    out[l, g, p, j*C + c] = w[l, p*KT + g*gk + j, c]."""
    ng = KT // gk
    w = w.reshape(L, 128, ng, gk, C)
    w = w.transpose(0, 2, 1, 3, 4)            # [L, ng, 128, gk, C]
    return np.ascontiguousarray(w.reshape(L, ng, 128, gk * C))


def _make_in_maps(x, base, mask, bitdelta):
    import ml_dtypes

    x = np.ascontiguousarray(x, dtype=np.float32)
    base = np.asarray(base, dtype=np.float32)
    mask = np.asarray(mask, dtype=np.float32)

    xT = np.ascontiguousarray(x.T).astype(ml_dtypes.bfloat16)    # [D, B]

    base16 = base.astype(ml_dtypes.bfloat16)
    mask8 = mask.astype(ml_dtypes.float8_e4m3)

    in_maps = []
    for c in range(NCORES):
        sl = slice(c * C, (c + 1) * C)
        in_maps.append({
            "xT0": xT,
            "base_sh": _shard_weight(base16[:, :, sl], GKB),
            "mask_sh": _shard_weight(mask8[:, :, sl], GKM),
        })
    return in_maps


def _run(x, base, mask, bitdelta, trace=False):
    nc = _get_nc(np.asarray(bitdelta, dtype=np.float32))
    in_maps = _make_in_maps(x, base, mask, bitdelta)
    res = run_bass_kernel_spmd(
        nc, in_maps, core_ids=list(range(NCORES)), trace=trace
    )
    y = np.concatenate([res.results[c]["out"] for c in range(NCORES)], axis=1)
    return y, res


def kernel(x, base, mask, bitdelta):
    y, _ = _run(x, base, mask, bitdelta)
    return y


# revision 8
# speedup vs baseline: 1.2735x; 1.2735x over previous
"""Trainium2 Bass kernel for a 4-layer compressed model:

    for l in range(4):  x = x @ (base[l] + bitdelta[l] * mask[l])

x: [16, 4096] f32, base/mask: [4, 4096, 4096] f32, bitdelta: [4] f32.

Sharding (8 cores, tensor parallel on weight columns):
  core c owns columns [c*512, (c+1)*512) of every layer's weight.

Key ideas:
  * Low-precision streaming: base is cast to bf16 on the host (quant
    noise ~0.2% of base, itself ~25x smaller than bd*mask); mask is
    +/-1 exactly, which fp8e4m3 represents exactly. Activations ride
    in bf16. HBM traffic per core: 64 MiB (f32) -> 24 MiB. bitdelta
    values are baked into the program as immediates (compiled after
    inputs are known; cache keyed on them).
  * The dense weight W = base + bd*mask is reconstructed ON-CHIP by the
    DVE (one scalar_tensor_tensor per 1 MiB chunk) as the chunks land.
    The combine depends only on the weight stream — never on gathered
    activations — so it always runs ahead, and the PE does a single
    matmul per k-tile (32/layer).
  * Queue discipline: ALL weight DMAs are issued up front on the sync
    queue (nothing gather-dependent ever blocks the stream); activation
    staging (y^T to DRAM) and x^T reloads ride the scalar queue; the
    collectives ride gpsimd. This keeps the weight stream saturated
    through every gather wait (3 layers of combined weights buffer in
    SBUF).
  * Between layers the [16,512] local result is PE-transposed to
    [512,16] bf16 and AllGather'd on the partition axis into the next
    layer's x^T — exactly the lhsT layout the next matmuls need.
  * PE warmers: a chain of throwaway matmuls anchored on yt_sb (ready
    exactly when the gather is triggered) runs through each gather
    window, holding the HAM clock gate at 2.4 GHz. Cold restarts both
    slow the next layer 2x and skew cores apart, and a collective
    completes only when the slowest rank arrives.

Memory-bound: each core streams 24 MiB of weights; roofline ~70 us.
"""

import numpy as np

import concourse.bass as bass
import concourse.mybir as mybir
import concourse.tile as tile
from concourse import bacc
from concourse.bass_utils import run_bass_kernel_spmd
from concourse.masks import make_identity

L = 4
D = 4096
B = 16
NCORES = 8
C = D // NCORES          # 512 columns per core
KT = D // 128            # 32 contraction tiles of 128
GKB = 8                  # k-tiles per base DMA (1 MiB bf16 transfers)
NGB = KT // GKB          # 4 base DMAs per layer
GKM = 16                 # k-tiles per mask DMA (1 MiB fp8 transfers)
NGM = KT // GKM          # 2 mask DMAs per layer
XCH = 4                  # x^T load chunks per layer
KXC = KT // XCH          # k-tiles per x chunk
CT = C // 128            # 4 transpose chunks
WBUFS_B = 6              # raw base tiles in flight
WBUFS_M = 3              # raw mask tiles in flight
WBUFS_C = 12             # combined tiles in flight (12 MiB, 3 layers)
NWARM = 32               # PE-warmer matmuls per gather window

F32 = mybir.dt.float32
BF16 = mybir.dt.bfloat16
FP8 = mybir.dt.float8e4
ALU = mybir.AluOpType

_cache = {}


def build(bd_vals):
    nc = bacc.Bacc(
        "TRN2",
        target_bir_lowering=False,
        debug=False,
        num_devices=NCORES,
    )

    # x^T in natural [4096, 16] order; row d = p*KT + k maps to SBUF
    # partition p, matmul index k — so the load is partition-contiguous.
    xT0 = nc.dram_tensor("xT0", [D, B], BF16, kind="ExternalInput")
    # weight shards, pre-permuted on host: [l, g, p, j*C+c] = W_l[p*KT+g*GK+j,
    # c]; each [128, GK*C] block is 1 MiB contiguous.
    base_sh = nc.dram_tensor("base_sh", [L, NGB, 128, GKB * C], BF16,
                             kind="ExternalInput")
    mask_sh = nc.dram_tensor("mask_sh", [L, NGM, 128, GKM * C], FP8,
                             kind="ExternalInput")
    out = nc.dram_tensor("out", [B, C], F32, kind="ExternalOutput")

    rg = [list(range(NCORES))]

    def load_xt_chunks(xpool, src):
        """Load x^T [D, B] into 4 SBUF chunk tiles of 8 k-tiles each."""
        chunks = []
        for xc in range(XCH):
            xt = xpool.tile([128, KXC * B], BF16, tag=f"xt{xc}")
            nc.scalar.dma_start(
                xt[:, :].rearrange("p (k b) -> p k b", k=KXC),
                src[:, :].rearrange("(p k) b -> p k b", p=128)
                [:, xc * KXC:(xc + 1) * KXC],
            )
            chunks.append(xt)
        return chunks

    with tile.TileContext(nc) as tc:
        with (
            tc.tile_pool(name="wb", bufs=WBUFS_B) as bpool,
            tc.tile_pool(name="wm", bufs=WBUFS_M) as mpool,
            tc.tile_pool(name="wc", bufs=WBUFS_C) as wcpool,
            tc.tile_pool(name="xp", bufs=2) as xpool,
            tc.tile_pool(name="sp", bufs=2) as spool,
            tc.tile_pool(name="const", bufs=1) as cpool,
            tc.tile_pool(name="acc", bufs=2, space="PSUM") as psum,
            tc.tile_pool(name="tp", bufs=4, space="PSUM") as tpsum,
            tc.tile_pool(name="warm", bufs=1, space="PSUM") as wpsum,
            tc.tile_pool(name="dram", bufs=2, space="DRAM") as dram,
        ):
            ident = cpool.tile([B, B], F32, tag="ident")
            make_identity(nc, ident[:, :])

            # Issue the whole weight stream + on-chip combines up front.
            # The sync queue carries only weight DMAs, so it never blocks
            # on activations; DVE carries only the combines.
            wcs = []
            for l in range(L):
                bd = float(bd_vals[l])
                wms = []
                for g in range(NGM):
                    wm = mpool.tile([128, GKM * C], FP8, tag="wm")
                    nc.sync.dma_start(wm[:, :], mask_sh[l, g])
                    wms.append(wm)
                wcs.append([])
                for g in range(NGB):
                    wb = bpool.tile([128, GKB * C], BF16, tag="wb")
                    nc.sync.dma_start(wb[:, :], base_sh[l, g])
                    wc = wcpool.tile([128, GKB * C], BF16, tag="wc")
                    half = (g % 2) * (GKB * C)
                    nc.vector.scalar_tensor_tensor(
                        out=wc[:, :],
                        in0=wms[g // 2][:, half:half + GKB * C],
                        scalar=bd,
                        in1=wb[:, :],
                        op0=ALU.mult,
                        op1=ALU.add,
                    )
                    wcs[l].append(wc)

            xts = load_xt_chunks(xpool, xT0)

            for l in range(L):
                acc = psum.tile([B, C], F32, tag="acc")
                for k in range(KT):
                    xc, kk = k // KXC, k % KXC
                    j = k % GKB
                    nc.tensor.matmul(
                        acc[:, :],
                        xts[xc][:, kk * B:(kk + 1) * B],
                        wcs[l][k // GKB][:, j * C:(j + 1) * C],
                        start=(k == 0),
                        stop=(k == KT - 1),
                    )

                y_sb = spool.tile([B, C], F32, tag="y")
                nc.scalar.copy(y_sb[:, :], acc[:, :])

                if l == L - 1:
                    nc.scalar.dma_start(out[:, :], y_sb[:, :])
                else:
                    # y [16, 512] -> y^T [512, 16] via 4 PE transposes,
                    # then AllGather into the next layer's x^T [4096, 16].
                    yt_sb = spool.tile([128, CT * B], BF16, tag="yt")
                    for cc in range(CT):
                        pt = tpsum.tile([128, B], F32, tag="pt")
                        nc.tensor.transpose(
                            pt[:, :],
                            y_sb[:, cc * 128:(cc + 1) * 128],
                            ident[:, :],
                        )
                        nc.scalar.copy(
                            yt_sb[:, cc * B:(cc + 1) * B], pt[:, :]
                        )
                    ytb = dram.tile([C, B], BF16, tag="ytb")
                    nc.scalar.dma_start(
                        ytb[:, :].rearrange("(cc p) b -> p cc b", p=128),
                        yt_sb[:, :].rearrange("p (cc b) -> p cc b", cc=CT),
                    )
                    xt_full = dram.tile([D, B], BF16, tag="xtf",
                                        addr_space="Shared")
                    nc.gpsimd.collective_compute(
                        "AllGather",
                        ALU.bypass,
                        replica_groups=rg,
                        ins=[ytb.opt()],
                        outs=[xt_full.opt()],
                    )

                    # PE warmers: anchored on yt_sb (ready right at gather
                    # trigger), they run back-to-back through the gather
                    # window on next-layer weights already in SBUF.
                    warm_ps = wpsum.tile([B, C], F32, tag="warm")
                    wsrc = wcs[l + 1][0]
                    for i in range(NWARM):
                        nc.tensor.matmul(
                            warm_ps[:, :],
                            yt_sb[:, :B],
                            wsrc[:, :C],
                            start=(i == 0),
                            stop=(i == NWARM - 1),
                        )

                    xts = load_xt_chunks(xpool, xt_full)

    nc.compile()
    return nc


def _get_nc(bd_vals):
    key = tuple(float(v) for v in bd_vals)
    if _cache.get("key") != key:
        _cache["nc"] = build(bd_vals)
        _cache["key"] = key
    return _cache["nc"]


def _shard_weight(w, gk):
    """[L, D, C] column shard -> [L, KT//gk, 128, gk*C] with
    out[l, g, p, j*C + c] = w[l, p*KT + g*gk + j, c]."""
    ng = KT // gk
    w = w.reshape(L, 128, ng, gk, C)
    w = w.transpose(0, 2, 1, 3, 4)            # [L, ng, 128, gk, C]
    return np.ascontiguousarray(w.reshape(L, ng, 128, gk * C))


def _make_in_maps(x, base, mask, bitdelta):
    import ml_dtypes

    x = np.ascontiguousarray(x, dtype=np.float32)
    base = np.asarray(base, dtype=np.float32)
    mask = np.asarray(mask, dtype=np.float32)

    xT = np.ascontiguousarray(x.T).astype(ml_dtypes.bfloat16)    # [D, B]

    base16 = base.astype(ml_dtypes.bfloat16)
    mask8 = mask.astype(ml_dtypes.float8_e4m3)

    in_maps = []
    for c in range(NCORES):
        sl = slice(c * C, (c + 1) * C)
        in_maps.append({
            "xT0": xT,
            "base_sh": _shard_weight(base16[:, :, sl], GKB),
            "mask_sh": _shard_weight(mask8[:, :, sl], GKM),
        })
    return in_maps


def _run(x, base, mask, bitdelta, trace=False):
    nc = _get_nc(np.asarray(bitdelta, dtype=np.float32))
    in_maps = _make_in_maps(x, base, mask, bitdelta)
    res = run_bass_kernel_spmd(
        nc, in_maps, core_ids=list(range(NCORES)), trace=trace
    )
    y = np.concatenate([res.results[c]["out"] for c in range(NCORES)], axis=1)
    return y, res


def kernel(x, base, mask, bitdelta):
    y, _ = _run(x, base, mask, bitdelta)
    return y
